# revision 36
# baseline (speedup 1.0000x reference)
"""Trainium2 Bass kernel for DeepAngAEVComputer (angular AEV: per-triplet MLP
with weighted per-atom scatter-add).

Contract: kernel(**inputs) takes the FULL unsharded inputs (B=8 molecules) and
returns the FULL [8, 32, 256] output.  Internally the batch axis is sharded
one molecule per NeuronCore across 8 cores (data parallel, no collectives).

Per-core layout
---------------
32 atoms x 496 pairs are padded to 512 pairs -> T = 16384 triplet "tokens".
Token t = atom*512 + pair.  Tokens are placed in a strip layout:
    strip a = t // 4096  (4 strips of 8 atoms)
    col-block b = (t % 4096) // 32   (128 blocks)
    u = t % 32
Token-major maps are [128, 128] arrays M[32a+u, b].  The 9 triplet features are
computed element-wise on the vector engine into Fbuf[32a+u, 32b+f], and a
32x32-block StreamTranspose yields xfm[32a+f, 32b+u] = feature-major activations
(strip a's tokens contiguous along the free axis).  The MLP then runs
feature-major: two 64-feature tiles are paired on the 128 partitions so tanh
activations use all scalar-engine lanes.  The last layer (128->256) is computed
token-major (tokens on PSUM partitions) so the weighted scatter-add over
triplets becomes 128 PE matmuls with a [128, 32] one-hot*w stationary operand
accumulating into a persistent [32, 256] PSUM tile.
"""

import os
from contextlib import ExitStack

import ml_dtypes
import numpy as np

import concourse.bass as bass
import concourse.tile as tile
from concourse import bacc
from concourse import mybir
from concourse.bass_utils import run_bass_kernel_spmd

F32 = mybir.dt.float32
BF16 = mybir.dt.bfloat16
ALU = mybir.AluOpType
ACTF = mybir.ActivationFunctionType

CUTOFF = 3.5
EPS = 1e-7
CLIP_MIN = 1e-10
PI = float(np.pi)

B = 8
NA = 32           # atoms per molecule
NPAIR = 496       # real pairs (32 choose 2)
PP = 512          # padded pairs per atom
T = NA * PP       # 16384 tokens per core
NBLK = 128        # 32-token col blocks
NST = 8           # MLP supertiles (2048 tokens each)

# matmul compute dtype: float32r = fp32 storage, relaxed-precision PE mode
# (1 col/cycle at N>=256 instead of fp32's 4 cycles/col).
_MM_DT_NAME = os.environ.get("AEV_MM_DT", "float16")
MM_DT = {"float32r": mybir.dt.float32r, "float32": F32,
         "bfloat16": BF16, "float16": mybir.dt.float16}[_MM_DT_NAME]


def _mm(ap):
    return ap


# --------------------------------------------------------------------------
# host-side input preparation
# --------------------------------------------------------------------------

def _tok_layout(V):
    """[32, 512] per-(atom, pair) values -> [128, 128] token-major map."""
    # token t = atom*512 + pair; map[32a+u, b] = V.flat[4096a + 32b + u]
    return np.ascontiguousarray(
        V.reshape(4, 128, 32).transpose(0, 2, 1).reshape(128, 128)
    )


def _onehot_np():
    # oh[32a+u, 32b+i] = 1 if atom(4096a + 32b + u) == i, atom = 8a + b//16
    a = np.arange(4)
    b = np.arange(128)
    atom = 8 * a[:, None] + b[None, :] // 16                   # [4, 128]
    oh = (atom[:, None, :, None] == np.arange(32)[None, None, None, :])
    oh = np.broadcast_to(oh, (4, 32, 128, 32))                 # [a, u, b, i]
    return np.ascontiguousarray(oh.reshape(128, 4096).astype(np.float32))


_JI, _KI = np.triu_indices(NA, k=1)


def make_core_inputs(D1, S1, Ws, bs):
    """Build one core's input map from its [32,32] distances + [32] species."""
    D1 = np.asarray(D1, np.float32)
    S1 = np.asarray(S1, np.float32)

    def pad(vals, fill):
        out = np.full((NA, PP), fill, np.float32)
        out[:, :NPAIR] = vals
        return out

    # pad with Rij=Rik=5.0 (> cutoff -> mask 0), benign Rjk/z values
    rij = pad(D1[:, _JI], 5.0)
    rik = pad(D1[:, _KI], 5.0)
    rjk = pad(np.broadcast_to(D1[_JI, _KI][None, :], (NA, NPAIR)), 1.0)
    zi = pad(np.broadcast_to(S1[:, None], (NA, NPAIR)), 1.0)
    zj = pad(np.broadcast_to(S1[_JI][None, :], (NA, NPAIR)), 1.0)
    zk = pad(np.broadcast_to(S1[_KI][None, :], (NA, NPAIR)), 1.0)

    geom = np.concatenate([_tok_layout(v)
                           for v in (rij, rik, rjk, zi, zj, zk)], axis=1)

    # block-stacked stationary weights: one K=128 matmul computes both
    # pair-members (member 0 -> out rows 0-63, member 1 -> rows 64-127).
    wcols = []
    for a in (0, 1):                       # w0stack_a: strips a and a+2
        w0s = np.zeros((128, 128), np.float32)
        w0s[32 * a:32 * a + 9, 0:64] = Ws[0]
        w0s[32 * (a + 2):32 * (a + 2) + 9, 64:128] = Ws[0]
        wcols.append(w0s)
    for l in range(1, 5):                  # block-diag(W_l, W_l)
        wd = np.zeros((128, 128), np.float32)
        wd[0:64, 0:64] = Ws[l]
        wd[64:128, 64:128] = Ws[l]
        wcols.append(wd)
    for m in (0, 1):                       # w5pad_m: member m's rows only
        w5p = np.zeros((128, 128), np.float32)
        w5p[64 * m:64 * m + 64, :] = Ws[5]
        wcols.append(w5p)
    wcols.append(Ws[6])
    for l in range(5):
        wcols.append(np.concatenate([bs[l], bs[l]])[:, None])
    wcols.append(np.tile(np.asarray(bs[5])[:, None], (2, 1))[:128])
    # K=128 bias trick: ones @ (b6/128 replicated) adds b6 with a fully
    # busy PE array (keeps the HAM activity monitor at full clock).
    wcols.append(np.ones((128, 128), np.float32))
    wcols.append(np.tile(np.asarray(bs[6], np.float32)[None, :], (128, 2)))
    strip1 = (np.arange(128)[:, None] % 32 ==
              np.arange(32)[None, :]).astype(np.float32)
    wcols.append(strip1)
    wpack = np.concatenate(wcols, axis=1).astype(np.float32)
    return {
        "geom": np.ascontiguousarray(geom, np.float32),
        "onehot": _onehot_np().astype(ml_dtypes.bfloat16),
        "wpack": np.ascontiguousarray(wpack, np.float32),
    }


# name -> (shape, is_matmul_operand)
# name -> (shape, is_matmul_operand, np dtype)
INPUT_SPECS = {
    "geom": ([128, 768], False, "float32"),
    "onehot": ([128, 4096], False, "bfloat16"),
    "wpack": ([128, 1958], True, "float32"),
}

# column offsets into wpack (weights + per-partition biases)
_WOFF = {"w0s0": (0, 128), "w0s1": (128, 256), "w1d": (256, 384),
         "w2d": (384, 512), "w3d": (512, 640), "w4d": (640, 768),
         "w5p0": (768, 896), "w5p1": (896, 1024), "w6": (1024, 1280),
         "b0rep": (1280, 1281), "b1rep": (1281, 1282),
         "b2rep": (1282, 1283), "b3rep": (1283, 1284),
         "b4rep": (1284, 1285), "b5c": (1285, 1286),
         "ones128": (1286, 1414), "b6div2": (1414, 1926),
         "strip1": (1926, 1958)}


# --------------------------------------------------------------------------
# device kernel
# --------------------------------------------------------------------------

def build_kernel(ctx, tc, out_ap, ins):
    """Emit the per-core kernel.  ins: dict name -> DRAM AP; out_ap: [32,256]."""
    nc = tc.nc

    consts = ctx.enter_context(tc.tile_pool(name="consts", bufs=1))
    fmaps = ctx.enter_context(tc.tile_pool(name="fmaps", bufs=1))
    big = ctx.enter_context(tc.tile_pool(name="big", bufs=1))
    actp = ctx.enter_context(tc.tile_pool(name="actp", bufs=8))
    otmp = ctx.enter_context(tc.tile_pool(name="otmp", bufs=3))
    apool = ctx.enter_context(tc.tile_pool(name="apool", bufs=6))
    psp = ctx.enter_context(tc.tile_pool(name="psp", bufs=3, space="PSUM"))
    gap = ctx.enter_context(tc.tile_pool(name="gap", bufs=1, space="PSUM"))

    # ---- load constants / inputs (packed: few DMAs, few semaphores) ----
    packed = {}
    staged = {}
    for name, (shape, is_mm, dtname) in INPUT_SPECS.items():
        dt_ = {"float32": F32, "bfloat16": BF16}[dtname]
        t = consts.tile(shape, dt_, tag=name, name=name)
        nc.sync.dma_start(t[:], ins[name][:])
        staged[name] = t
        if is_mm and MM_DT is not F32:
            tr = consts.tile(shape, MM_DT, tag=name + "_r", name=name + "_r")
            nc.vector.tensor_copy(tr[:], t[:])
            t = tr
        packed[name] = t

    cb = {}
    for nm, (c0, c1) in _WOFF.items():
        # matmul operands from the MM_DT copy; ACT bias operands from f32
        is_bias = nm in ("b0rep", "b1rep", "b2rep", "b3rep", "b4rep", "b5c")
        srcbuf = staged["wpack"] if is_bias else packed["wpack"]
        cb[nm] = srcbuf[:, c0:c1]
    geom = packed["geom"]
    oh_full = packed["onehot"]

    # ---- feature maps ----
    def fm(tag):
        return fmaps.tile([128, 128], F32, tag=tag, name=tag)

    rij_f, rik_f, rjk_f = geom[:, 0:128], geom[:, 128:256], geom[:, 256:384]
    zi_f, zj_f, zk_f = geom[:, 384:512], geom[:, 512:640], geom[:, 640:768]

    halfpi = fmaps.tile([128, 1], F32, tag="halfpi", name="halfpi")
    nc.vector.memset(halfpi[:], PI / 2)
    nc.scalar.activation(halfpi[:], halfpi[:], ACTF.Tanh)
    nc.vector.memset(halfpi[:], PI / 2)
    fbuf = big.tile([128, 128, 32], F32, tag="fbuf")
    nc.gpsimd.memset(fbuf[:], 0.0)

    w_tm = fm("w_tm")
    M = {n: fm(n) for n in
         ("fci", "fck", "m1", "sq_ij", "sq_ik", "sq_jk", "p_ijik", "p_ijjk",
          "p_ikjk", "r_i", "r_j", "r_k", "tN", "c_i", "c_j", "c_k", "g0",
          "g1", "g2", "gs", "tq", "zs", "csum", "zp", "cp", "zc", "t4",
          "AA", "cs", "ch0", "ch1", "ch2", "ch3", "ch4", "ch5")}

    def features_half(h):
        """Geo+chem features for col-blocks [64h, 64h+64) on the DVE."""
        sl = slice(64 * h, 64 * h + 64)
        def s(nm):
            return M[nm][:, sl]
        rij, rik, rjk = rij_f[:, sl], rik_f[:, sl], rjk_f[:, sl]
        zi, zj, zk = zi_f[:, sl], zj_f[:, sl], zk_f[:, sl]

        def TT(out, a, b_, op):
            nc.vector.tensor_tensor(out=out, in0=a, in1=b_, op=op)
            return out

        # squares / pair products (squares on idle gpsimd, products on DVE)
        nc.gpsimd.tensor_tensor(out=s("sq_ij"), in0=rij, in1=rij, op=ALU.mult)
        nc.gpsimd.tensor_tensor(out=s("sq_ik"), in0=rik, in1=rik, op=ALU.mult)
        nc.gpsimd.tensor_tensor(out=s("sq_jk"), in0=rjk, in1=rjk, op=ALU.mult)
        TT(s("p_ijik"), rij, rik, ALU.mult)
        TT(s("p_ijjk"), rij, rjk, ALU.mult)
        TT(s("p_ikjk"), rik, rjk, ALU.mult)

        # Carnot cosines
        for rn, pn in (("r_i", "p_ijik"), ("r_j", "p_ijjk"), ("r_k", "p_ikjk")):
            nc.vector.tensor_scalar(out=s(rn), in0=s(pn), scalar1=2.0,
                                    scalar2=CLIP_MIN, op0=ALU.mult, op1=ALU.max)
            nc.vector.reciprocal(out=s(rn), in_=s(rn))
        TT(s("tN"), s("sq_ij"), s("sq_ik"), ALU.add)
        TT(s("tN"), s("tN"), s("sq_jk"), ALU.subtract)
        TT(s("c_i"), s("tN"), s("r_i"), ALU.mult)
        TT(s("tN"), s("sq_ij"), s("sq_jk"), ALU.add)
        TT(s("tN"), s("tN"), s("sq_ik"), ALU.subtract)
        TT(s("c_j"), s("tN"), s("r_j"), ALU.mult)
        TT(s("tN"), s("sq_ik"), s("sq_jk"), ALU.add)
        TT(s("tN"), s("tN"), s("sq_ij"), ALU.subtract)
        TT(s("c_k"), s("tN"), s("r_k"), ALU.mult)

        # geo features -> fbuf[:, :, 0:3]
        TT(s("g0"), rij, rik, ALU.add)
        TT(s("g1"), rjk, s("g0"), ALU.mult)
        TT(s("g0"), s("g0"), rjk, ALU.add)
        TT(s("g1"), s("g1"), s("p_ijik"), ALU.add)
        TT(s("g2"), s("p_ijik"), rjk, ALU.mult)
        TT(s("gs"), s("g0"), s("g0"), ALU.mult)
        TT(s("tq"), s("g1"), s("g1"), ALU.mult)
        TT(s("gs"), s("gs"), s("tq"), ALU.add)
        TT(s("tq"), s("g2"), s("g2"), ALU.mult)
        TT(s("gs"), s("gs"), s("tq"), ALU.add)
        nc.scalar.activation(s("gs"), s("gs"), ACTF.Sqrt)
        nc.vector.tensor_scalar(out=s("gs"), in0=s("gs"), scalar1=EPS,
                                scalar2=None, op0=ALU.add)
        nc.vector.reciprocal(out=s("gs"), in_=s("gs"))
        TT(fbuf[:, sl, 0], s("g0"), s("gs"), ALU.mult)
        TT(fbuf[:, sl, 1], s("g1"), s("gs"), ALU.mult)
        TT(fbuf[:, sl, 2], s("g2"), s("gs"), ALU.mult)

        # chem features -> fbuf[:, :, 3:9]
        nc.gpsimd.tensor_tensor(out=s("zs"), in0=zj, in1=zk, op=ALU.add)
        TT(s("csum"), s("c_j"), s("c_k"), ALU.add)
        nc.gpsimd.tensor_tensor(out=s("zp"), in0=zj, in1=zk, op=ALU.mult)
        TT(s("cp"), s("c_j"), s("c_k"), ALU.mult)
        TT(s("zc"), zj, s("c_k"), ALU.mult)
        TT(s("t4"), s("c_j"), zk, ALU.mult)
        TT(s("zc"), s("zc"), s("t4"), ALU.add)
        TT(s("AA"), s("zp"), s("cp"), ALU.subtract)
        TT(s("ch0"), zi, s("zs"), ALU.add)
        TT(s("ch1"), s("c_i"), s("csum"), ALU.add)
        TT(s("ch2"), zi, s("zs"), ALU.mult)
        TT(s("ch2"), s("ch2"), s("zp"), ALU.add)
        TT(s("t4"), s("c_i"), s("csum"), ALU.mult)
        TT(s("ch2"), s("ch2"), s("t4"), ALU.subtract)
        TT(s("ch2"), s("ch2"), s("cp"), ALU.subtract)
        TT(s("ch3"), zi, s("csum"), ALU.mult)
        TT(s("t4"), s("c_i"), s("zs"), ALU.mult)
        TT(s("ch3"), s("ch3"), s("t4"), ALU.add)
        TT(s("ch3"), s("ch3"), s("zc"), ALU.add)
        TT(s("ch4"), zi, s("AA"), ALU.mult)
        TT(s("t4"), s("c_i"), s("zc"), ALU.mult)
        TT(s("ch4"), s("ch4"), s("t4"), ALU.subtract)
        TT(s("ch5"), zi, s("zc"), ALU.mult)
        TT(s("t4"), s("c_i"), s("AA"), ALU.mult)
        TT(s("ch5"), s("ch5"), s("t4"), ALU.add)
        TT(s("cs"), s("ch0"), s("ch0"), ALU.mult)
        for i in range(1, 6):
            TT(s("t4"), s(f"ch{i}"), s(f"ch{i}"), ALU.mult)
            TT(s("cs"), s("cs"), s("t4"), ALU.add)
        nc.scalar.activation(s("cs"), s("cs"), ACTF.Sqrt)
        nc.vector.tensor_scalar(out=s("cs"), in0=s("cs"), scalar1=EPS,
                                scalar2=None, op0=ALU.add)
        nc.vector.reciprocal(out=s("cs"), in_=s("cs"))
        for i in range(6):
            TT(fbuf[:, sl, 3 + i], s(f"ch{i}"), s("cs"), ALU.mult)

    def wchain_half(h):
        """Cutoff weight w = fc(rij)*fc(rik)*mask on gpsimd + scalar.
        cos(pi*d/C) = sin(pi/2 - pi*d/C), d clamped to [0, C] to stay in the
        scalar engine's sin spline range (out-of-cutoff values are masked)."""
        sl = slice(64 * h, 64 * h + 64)
        rij, rik = rij_f[:, sl], rik_f[:, sl]
        fci, fck, m1 = M["fci"][:, sl], M["fck"][:, sl], M["m1"][:, sl]
        w = w_tm[:, sl]
        g = nc.vector
        g.tensor_scalar(out=fci, in0=rij, scalar1=CUTOFF, scalar2=None,
                        op0=ALU.min)
        g.tensor_scalar(out=fck, in0=rik, scalar1=CUTOFF, scalar2=None,
                        op0=ALU.min)
        nc.scalar.activation(fci, fci, ACTF.Sin, bias=halfpi[:, 0:1],
                             scale=-PI / CUTOFF)
        nc.scalar.activation(fck, fck, ACTF.Sin, bias=halfpi[:, 0:1],
                             scale=-PI / CUTOFF)
        g.tensor_scalar(out=fci, in0=fci, scalar1=0.5, scalar2=0.5,
                        op0=ALU.mult, op1=ALU.add)
        g.tensor_scalar(out=fck, in0=fck, scalar1=0.5, scalar2=0.5,
                        op0=ALU.mult, op1=ALU.add)
        g.tensor_tensor(out=w, in0=fci, in1=fck, op=ALU.mult)
        g.tensor_scalar(out=m1, in0=rij, scalar1=CUTOFF, scalar2=None,
                        op0=ALU.is_lt)
        g.scalar_tensor_tensor(out=m1, in0=rik, scalar=CUTOFF, in1=m1,
                               op0=ALU.is_lt, op1=ALU.mult)
        g.scalar_tensor_tensor(out=m1, in0=rij, scalar=0.0, in1=m1,
                               op0=ALU.is_gt, op1=ALU.mult)
        g.scalar_tensor_tensor(out=m1, in0=rik, scalar=0.0, in1=m1,
                               op0=ALU.is_gt, op1=ALU.mult)
        g.tensor_tensor(out=w, in0=w, in1=m1, op=ALU.mult)

    # ---- feature-major activations via 32x32 block transpose ----
    xfm_r = big.tile([128, 4096], MM_DT, tag="xfm_r")
    # xb3 columns are block-major: col = 128*bb + 32*a + u  (token 4096a+32bb+u)
    # so each col-block's 128 tokens are contiguous for the L6 stationary AP.
    xb3 = big.tile([128, 16384], MM_DT, tag="xb3")
    xb3_4 = xb3[:].rearrange("p (bb a u) -> p bb a u", a=4, u=32)

    ga = gap.tile([128, 256], F32, tag="ga")

    w6 = cb["w6"]
    ones128 = cb["ones128"]
    b6mul = cb["b6div2"]
    oh = oh_full

    def mm_pair(dst_ps, wt, src, tloc=None):
        for j in range(2):
            rhs = (xfm_r[:, 512 * (tloc + j):512 * (tloc + j) + 512]
                   if src is None else src[:, 512 * j:512 * j + 512])
            nc.tensor.matmul(dst_ps[:, 512 * j:512 * j + 512], _mm(wt[:]),
                             _mm(rhs), start=True, stop=True)

    def make_mlp_stages(s):
        """Stage callables for supertile s (tiles {2s, 2s+1, 2s+16, 2s+17}).

        Pair-members are packed via block-stacked stationary weights: one
        K=128 matmul computes member 0 into out rows 0-63 and member 1 into
        rows 64-127 (rows outside each member's block are zero weights).
        Split into stages so two supertiles + L6 work interleave on the PE
        (keeps it busy across each tanh dependency, which also keeps the
        HAM clock gate at full rate)."""
        tloc = (2 * s) % 8                 # within-strip tile index base
        st = {}

        def tanh_stage(ps, bname):
            dst = actp.tile([128, 1024], MM_DT, tag="h", name="h")
            nc.scalar.activation(dst[:], ps[:], ACTF.Tanh,
                                 bias=cb[bname][:, 0:1])
            return dst

        def st_l0():
            ps = psp.tile([128, 1024], F32, tag="ps", name="ps")
            w0s = cb["w0s0"] if s // 4 == 0 else cb["w0s1"]
            mm_pair(ps, w0s, None, tloc)
            st["xres"] = tanh_stage(ps, "b0rep")

        def st_l1():
            ps = psp.tile([128, 1024], F32, tag="ps", name="ps")
            mm_pair(ps, cb["w1d"], st["xres"])
            x1 = tanh_stage(ps, "b1rep")
            xb1 = actp.tile([128, 1024], MM_DT, tag="h", name="h")
            nc.vector.tensor_tensor(out=xb1[:], in0=x1[:], in1=st["xres"][:],
                                    op=ALU.add)
            st["xb1"] = xb1

        def st_l2():
            ps = psp.tile([128, 1024], F32, tag="ps", name="ps")
            mm_pair(ps, cb["w2d"], st["xb1"])
            st["x2"] = tanh_stage(ps, "b2rep")

        def st_l3():
            ps = psp.tile([128, 1024], F32, tag="ps", name="ps")
            mm_pair(ps, cb["w3d"], st["x2"])
            st["x3"] = tanh_stage(ps, "b3rep")

        def st_l4():
            ps = psp.tile([128, 1024], F32, tag="ps", name="ps")
            mm_pair(ps, cb["w4d"], st["x3"])
            x4 = tanh_stage(ps, "b4rep")
            xb2 = actp.tile([128, 1024], MM_DT, tag="h", name="h")
            nc.vector.tensor_tensor(out=xb2[:], in0=x4[:], in1=st["xb1"][:],
                                    op=ALU.add)
            st["xb2"] = xb2

        def st_l5(m):
            ps5 = psp.tile([128, 1024], F32, tag="ps", name="ps")
            mm_pair(ps5, cb["w5p0"] if m == 0 else cb["w5p1"], st["xb2"])
            a = (s // 4) + 2 * m
            nc.scalar.activation(
                xb3_4[:, 16 * tloc:16 * tloc + 32, a, :], ps5[:], ACTF.Tanh,
                bias=cb["b5c"][:, 0:1])

        return [st_l0, st_l1, st_l2, st_l3, st_l4,
                lambda: st_l5(0), lambda: st_l5(1)]

    def l6_quad(q):
        """Final layer + weighted accumulation for col-blocks 4q..4q+3."""
        ps6 = psp.tile([128, 1024], F32, tag="ps")
        for jj in range(2):
            nc.tensor.matmul(
                ps6[:, 512 * jj:512 * jj + 512],
                _mm(ones128[0:1, :]), _mm(b6mul[0:1, :]),
                start=True, stop=False, skip_group_check=True,
            )
        for i in range(4):
            b_ = 4 * q + i
            nc.tensor.matmul(
                ps6[:, 256 * i:256 * i + 256],
                _mm(xb3[:, 128 * b_:128 * b_ + 128]),
                _mm(w6[:]),
                start=False, stop=True, skip_group_check=True,
            )
        otm = otmp.tile([128, 1024], MM_DT, tag="otm")
        nc.scalar.activation(otm[:], ps6[:], ACTF.Tanh)
        ab = apool.tile([128, 4, 32], MM_DT, tag="ab", name="ab")
        oh4 = oh[:, 128 * q:128 * q + 128].rearrange("p (b u) -> p b u", b=4)
        wq = w_tm[:, 4 * q:4 * q + 4]
        w_bc = bass.AP(tensor=wq.tensor, offset=wq.offset,
                       ap=[list(wq.ap[0]), [int(wq.ap[1][0]), 4], [0, 32]])
        nc.vector.tensor_tensor(out=ab[:], in0=oh4, in1=w_bc, op=ALU.mult)
        for i in range(4):
            b_ = 4 * q + i
            nc.tensor.matmul(
                ga[32 * i:32 * i + 32, :], _mm(ab[:, i, :]),
                _mm(otm[:, 256 * i:256 * i + 256]),
                start=(q == 0), stop=(q == 31), skip_group_check=True,
                tile_position=(0, 32 * i),
            )

    # Feature pipeline: compute half the col-blocks, transpose them in
    # place (32x32 blocks fill-then-drain, safe in place), convert to the
    # matmul dtype, then the MLP starts while the second half computes.
    # The cutoff-weight chain runs on gpsimd/scalar off the critical path.
    for h in range(2):
        features_half(h)
        for c in range(4 * h, 4 * h + 4):
            nc.vector.transpose(out=fbuf[:, 16 * c:16 * c + 16, :],
                                in_=fbuf[:, 16 * c:16 * c + 16, :])
            nc.vector.tensor_copy(xfm_r[:, 512 * c:512 * c + 512],
                                  fbuf[:, 16 * c:16 * c + 16, :])
        wchain_half(h)

    # Interleave: each group emits two supertile stage-chains round-robin
    # with the PREVIOUS group's L6 quads, so the PE always has independent
    # matmul work while tanh stages drain.
    pending_quads = []
    for g in range(4):
        sa = make_mlp_stages(g)
        sb = make_mlp_stages(g + 4)
        quads = list(pending_quads)
        for i in range(7):
            sa[i]()
            sb[i]()
            if i < len(quads):
                l6_quad(quads[i])
        for q in quads[7:]:
            l6_quad(q)
        pending_quads = list(range(8 * g, 8 * g + 8))
    for q in pending_quads:
        l6_quad(q)

    # ---- sum the 4 GA strips, normalize rows, write out ----
    ga4sb = fmaps.tile([128, 256], F32, tag="ga4sb", name="ga4sb")
    nc.scalar.activation(ga4sb[:], ga[:], ACTF.Copy)
    gsum = gap.tile([32, 256], F32, tag="gsum", name="gsum")
    nc.tensor.matmul(gsum[:, :], staged["wpack"][:, 1926:1958],
                     ga4sb[:], start=True, stop=True)
    ga = gsum
    sqbuf = fmaps.tile([32, 256], F32, tag="sqbuf")
    ss = fmaps.tile([32, 1], F32, tag="ss")
    nc.scalar.activation(sqbuf[:], ga[:], ACTF.Square, accum_out=ss[:, 0:1])
    nc.scalar.activation(ss[:], ss[:], ACTF.Sqrt)
    nc.vector.tensor_scalar(out=ss[:], in0=ss[:], scalar1=EPS, scalar2=None,
                            op0=ALU.add)
    nc.vector.reciprocal(out=ss[:], in_=ss[:])
    outs = fmaps.tile([32, 256], F32, tag="outs")
    nc.vector.tensor_scalar(out=outs[:], in0=ga[:], scalar1=ss[:, 0:1],
                            scalar2=None, op0=ALU.mult)
    nc.sync.dma_start(out_ap[:], outs[:])


# --------------------------------------------------------------------------
# module build + run
# --------------------------------------------------------------------------

_BUILT = {}


def build_bass():
    key = _MM_DT_NAME
    if key in _BUILT:
        return _BUILT[key]
    nc = bacc.Bacc(trn_type="TRN2", target_bir_lowering=False, debug=False)
    ins = {
        name: nc.dram_tensor(name, shape,
                             {"float32": F32, "bfloat16": BF16}[dtname],
                             kind="ExternalInput").ap()
        for name, (shape, is_mm, dtname) in INPUT_SPECS.items()
    }
    out = nc.dram_tensor("out", [NA, 256], F32, kind="ExternalOutput").ap()
    with tile.TileContext(nc) as tc:
        with ExitStack() as ctx:
            build_kernel(ctx, tc, out, ins)
    nc.finalize()
    _BUILT[key] = nc
    return nc


def _run(inputs, **spmd_kwargs):
    D = np.asarray(inputs["distance_matrices"], np.float32)
    S = np.asarray(inputs["num_species_batch"], np.float32)
    Ws = [np.asarray(inputs[f"W{i}"], np.float32) for i in range(7)]
    bs = [np.asarray(inputs[f"b{i}"], np.float32) for i in range(7)]
    nc = build_bass()
    in_maps = [make_core_inputs(D[c], S[c], Ws, bs) for c in range(B)]
    res = run_bass_kernel_spmd(nc, in_maps, core_ids=list(range(B)),
                               **spmd_kwargs)
    out = np.stack([res.results[c]["out"] for c in range(B)], axis=0)
    return out.astype(np.float32), res


def kernel(**inputs):
    out, _ = _run(inputs)
    return out


# revision 37
# speedup vs baseline: 1.0490x; 1.0490x over previous
"""Trainium2 Bass kernel for DeepAngAEVComputer (angular AEV: per-triplet MLP
with weighted per-atom scatter-add).

Contract: kernel(**inputs) takes the FULL unsharded inputs (B=8 molecules) and
returns the FULL [8, 32, 256] output.  Internally the batch axis is sharded
one molecule per NeuronCore across 8 cores (data parallel, no collectives).

Per-core layout
---------------
32 atoms x 496 pairs are padded to 512 pairs -> T = 16384 triplet "tokens".
Token t = atom*512 + pair.  Tokens are placed in a strip layout:
    strip a = t // 4096  (4 strips of 8 atoms)
    col-block b = (t % 4096) // 32   (128 blocks)
    u = t % 32
Token-major maps are [128, 128] arrays M[32a+u, b].  The 9 triplet features are
computed element-wise on the vector engine into Fbuf[32a+u, 32b+f], and a
32x32-block StreamTranspose yields xfm[32a+f, 32b+u] = feature-major activations
(strip a's tokens contiguous along the free axis).  The MLP then runs
feature-major: two 64-feature tiles are paired on the 128 partitions so tanh
activations use all scalar-engine lanes.  The last layer (128->256) is computed
token-major (tokens on PSUM partitions) so the weighted scatter-add over
triplets becomes 128 PE matmuls with a [128, 32] one-hot*w stationary operand
accumulating into a persistent [32, 256] PSUM tile.
"""

import os
from contextlib import ExitStack

import ml_dtypes
import numpy as np

import concourse.bass as bass
import concourse.tile as tile
from concourse import bacc
from concourse import mybir
from concourse.bass_utils import run_bass_kernel_spmd

F32 = mybir.dt.float32
BF16 = mybir.dt.bfloat16
ALU = mybir.AluOpType
ACTF = mybir.ActivationFunctionType

CUTOFF = 3.5
EPS = 1e-7
CLIP_MIN = 1e-10
PI = float(np.pi)

B = 8
NA = 32           # atoms per molecule
NPAIR = 496       # real pairs (32 choose 2)
PP = 512          # padded pairs per atom
T = NA * PP       # 16384 tokens per core
NBLK = 128        # 32-token col blocks
NST = 8           # MLP supertiles (2048 tokens each)

# matmul compute dtype: float32r = fp32 storage, relaxed-precision PE mode
# (1 col/cycle at N>=256 instead of fp32's 4 cycles/col).
_MM_DT_NAME = os.environ.get("AEV_MM_DT", "float16")
MM_DT = {"float32r": mybir.dt.float32r, "float32": F32,
         "bfloat16": BF16, "float16": mybir.dt.float16}[_MM_DT_NAME]


def _mm(ap):
    return ap


# --------------------------------------------------------------------------
# host-side input preparation
# --------------------------------------------------------------------------

def _tok_layout(V):
    """[32, 512] per-(atom, pair) values -> [128, 128] token-major map."""
    # token t = atom*512 + pair; map[32a+u, b] = V.flat[4096a + 32b + u]
    return np.ascontiguousarray(
        V.reshape(4, 128, 32).transpose(0, 2, 1).reshape(128, 128)
    )


def _onehot_np():
    # oh[32a+u, 32b+i] = 1 if atom(4096a + 32b + u) == i, atom = 8a + b//16
    a = np.arange(4)
    b = np.arange(128)
    atom = 8 * a[:, None] + b[None, :] // 16                   # [4, 128]
    oh = (atom[:, None, :, None] == np.arange(32)[None, None, None, :])
    oh = np.broadcast_to(oh, (4, 32, 128, 32))                 # [a, u, b, i]
    return np.ascontiguousarray(oh.reshape(128, 4096).astype(np.float32))


_JI, _KI = np.triu_indices(NA, k=1)


def make_core_inputs(D1, S1, Ws, bs):
    """Build one core's input map from its [32,32] distances + [32] species."""
    D1 = np.asarray(D1, np.float32)
    S1 = np.asarray(S1, np.float32)

    def pad(vals, fill):
        out = np.full((NA, PP), fill, np.float32)
        out[:, :NPAIR] = vals
        return out

    # pad with Rij=Rik=5.0 (> cutoff -> mask 0), benign Rjk/z values
    rij = pad(D1[:, _JI], 5.0)
    rik = pad(D1[:, _KI], 5.0)
    rjk = pad(np.broadcast_to(D1[_JI, _KI][None, :], (NA, NPAIR)), 1.0)
    zi = pad(np.broadcast_to(S1[:, None], (NA, NPAIR)), 1.0)
    zj = pad(np.broadcast_to(S1[_JI][None, :], (NA, NPAIR)), 1.0)
    zk = pad(np.broadcast_to(S1[_KI][None, :], (NA, NPAIR)), 1.0)

    geom = np.concatenate([_tok_layout(v)
                           for v in (rij, rik, rjk, zi, zj, zk)], axis=1)

    # block-stacked stationary weights: one K=128 matmul computes both
    # pair-members (member 0 -> out rows 0-63, member 1 -> rows 64-127).
    wcols = []
    for a in (0, 1):                       # w0stack_a: strips a and a+2
        w0s = np.zeros((128, 128), np.float32)
        w0s[32 * a:32 * a + 9, 0:64] = Ws[0]
        w0s[32 * (a + 2):32 * (a + 2) + 9, 64:128] = Ws[0]
        wcols.append(w0s)
    for l in range(1, 5):                  # block-diag(W_l, W_l)
        wd = np.zeros((128, 128), np.float32)
        wd[0:64, 0:64] = Ws[l]
        wd[64:128, 64:128] = Ws[l]
        wcols.append(wd)
    for m in (0, 1):                       # w5pad_m: member m's rows only
        w5p = np.zeros((128, 128), np.float32)
        w5p[64 * m:64 * m + 64, :] = Ws[5]
        wcols.append(w5p)
    wcols.append(Ws[6])
    for l in range(5):
        wcols.append(np.concatenate([bs[l], bs[l]])[:, None])
    wcols.append(np.tile(np.asarray(bs[5])[:, None], (2, 1))[:128])
    # K=128 bias trick: ones @ (b6/128 replicated) adds b6 with a fully
    # busy PE array (keeps the HAM activity monitor at full clock).
    wcols.append(np.ones((128, 128), np.float32))
    wcols.append(np.tile(np.asarray(bs[6], np.float32)[None, :], (128, 2)))
    strip1 = (np.arange(128)[:, None] % 32 ==
              np.arange(32)[None, :]).astype(np.float32)
    wcols.append(strip1)
    wpack = np.concatenate(wcols, axis=1).astype(np.float32)
    return {
        "geom": np.ascontiguousarray(geom, np.float32),
        "onehot": _onehot_np().astype(ml_dtypes.bfloat16),
        "wpack": np.ascontiguousarray(wpack, np.float32),
    }


# name -> (shape, is_matmul_operand)
# name -> (shape, is_matmul_operand, np dtype)
INPUT_SPECS = {
    "geom": ([128, 768], False, "float32"),
    "onehot": ([128, 4096], False, "bfloat16"),
    "wpack": ([128, 1958], True, "float32"),
}

# column offsets into wpack (weights + per-partition biases)
_WOFF = {"w0s0": (0, 128), "w0s1": (128, 256), "w1d": (256, 384),
         "w2d": (384, 512), "w3d": (512, 640), "w4d": (640, 768),
         "w5p0": (768, 896), "w5p1": (896, 1024), "w6": (1024, 1280),
         "b0rep": (1280, 1281), "b1rep": (1281, 1282),
         "b2rep": (1282, 1283), "b3rep": (1283, 1284),
         "b4rep": (1284, 1285), "b5c": (1285, 1286),
         "ones128": (1286, 1414), "b6div2": (1414, 1926),
         "strip1": (1926, 1958)}


# --------------------------------------------------------------------------
# device kernel
# --------------------------------------------------------------------------

def build_kernel(ctx, tc, out_ap, ins):
    """Emit the per-core kernel.  ins: dict name -> DRAM AP; out_ap: [32,256]."""
    nc = tc.nc

    consts = ctx.enter_context(tc.tile_pool(name="consts", bufs=1))
    fmaps = ctx.enter_context(tc.tile_pool(name="fmaps", bufs=1))
    big = ctx.enter_context(tc.tile_pool(name="big", bufs=1))
    actp = ctx.enter_context(tc.tile_pool(name="actp", bufs=8))
    otmp = ctx.enter_context(tc.tile_pool(name="otmp", bufs=2))
    apool = ctx.enter_context(tc.tile_pool(name="apool", bufs=4))
    psp = ctx.enter_context(tc.tile_pool(name="psp", bufs=3, space="PSUM"))
    gap = ctx.enter_context(tc.tile_pool(name="gap", bufs=1, space="PSUM"))

    # ---- load constants / inputs (packed: few DMAs, few semaphores) ----
    packed = {}
    staged = {}
    for name, (shape, is_mm, dtname) in INPUT_SPECS.items():
        dt_ = {"float32": F32, "bfloat16": BF16}[dtname]
        t = consts.tile(shape, dt_, tag=name, name=name)
        nc.sync.dma_start(t[:], ins[name][:])
        staged[name] = t
        if is_mm and MM_DT is not F32:
            tr = consts.tile(shape, MM_DT, tag=name + "_r", name=name + "_r")
            nc.vector.tensor_copy(tr[:], t[:])
            t = tr
        packed[name] = t

    cb = {}
    for nm, (c0, c1) in _WOFF.items():
        # matmul operands from the MM_DT copy; ACT bias operands from f32
        is_bias = nm in ("b0rep", "b1rep", "b2rep", "b3rep", "b4rep", "b5c")
        srcbuf = staged["wpack"] if is_bias else packed["wpack"]
        cb[nm] = srcbuf[:, c0:c1]
    geom = packed["geom"]
    oh_full = packed["onehot"]

    # ---- feature maps ----
    def fm(tag):
        return fmaps.tile([128, 128], F32, tag=tag, name=tag)

    rij_f, rik_f, rjk_f = geom[:, 0:128], geom[:, 128:256], geom[:, 256:384]
    zi_f, zj_f, zk_f = geom[:, 384:512], geom[:, 512:640], geom[:, 640:768]

    halfpi = fmaps.tile([128, 1], F32, tag="halfpi", name="halfpi")
    nc.vector.memset(halfpi[:], PI / 2)
    nc.scalar.activation(halfpi[:], halfpi[:], ACTF.Tanh)
    nc.vector.memset(halfpi[:], PI / 2)
    fbuf = big.tile([128, 128, 32], F32, tag="fbuf")
    nc.gpsimd.memset(fbuf[:], 0.0)

    w_tm = fm("w_tm")
    M = {n: fm(n) for n in
         ("fci", "fck", "m1", "sq_ij", "sq_ik", "sq_jk", "p_ijik", "p_ijjk",
          "p_ikjk", "r_i", "r_j", "r_k", "tN", "c_i", "c_j", "c_k", "g0",
          "g1", "g2", "gs", "tq", "zs", "csum", "zp", "cp", "zc", "t4",
          "AA", "cs", "ch0", "ch1", "ch2", "ch3", "ch4", "ch5")}

    def features_half(h):
        """Geo+chem features for col-blocks [64h, 64h+64) on the DVE."""
        sl = slice(64 * h, 64 * h + 64)
        def s(nm):
            return M[nm][:, sl]
        rij, rik, rjk = rij_f[:, sl], rik_f[:, sl], rjk_f[:, sl]
        zi, zj, zk = zi_f[:, sl], zj_f[:, sl], zk_f[:, sl]

        def TT(out, a, b_, op):
            nc.vector.tensor_tensor(out=out, in0=a, in1=b_, op=op)
            return out

        # squares / pair products (squares on idle gpsimd, products on DVE)
        nc.gpsimd.tensor_tensor(out=s("sq_ij"), in0=rij, in1=rij, op=ALU.mult)
        nc.gpsimd.tensor_tensor(out=s("sq_ik"), in0=rik, in1=rik, op=ALU.mult)
        nc.gpsimd.tensor_tensor(out=s("sq_jk"), in0=rjk, in1=rjk, op=ALU.mult)
        TT(s("p_ijik"), rij, rik, ALU.mult)
        TT(s("p_ijjk"), rij, rjk, ALU.mult)
        TT(s("p_ikjk"), rik, rjk, ALU.mult)

        # Carnot cosines
        for rn, pn in (("r_i", "p_ijik"), ("r_j", "p_ijjk"), ("r_k", "p_ikjk")):
            nc.vector.tensor_scalar(out=s(rn), in0=s(pn), scalar1=2.0,
                                    scalar2=CLIP_MIN, op0=ALU.mult, op1=ALU.max)
            nc.vector.reciprocal(out=s(rn), in_=s(rn))
        TT(s("tN"), s("sq_ij"), s("sq_ik"), ALU.add)
        TT(s("tN"), s("tN"), s("sq_jk"), ALU.subtract)
        TT(s("c_i"), s("tN"), s("r_i"), ALU.mult)
        TT(s("tN"), s("sq_ij"), s("sq_jk"), ALU.add)
        TT(s("tN"), s("tN"), s("sq_ik"), ALU.subtract)
        TT(s("c_j"), s("tN"), s("r_j"), ALU.mult)
        TT(s("tN"), s("sq_ik"), s("sq_jk"), ALU.add)
        TT(s("tN"), s("tN"), s("sq_ij"), ALU.subtract)
        TT(s("c_k"), s("tN"), s("r_k"), ALU.mult)

        # geo features -> fbuf[:, :, 0:3]
        TT(s("g0"), rij, rik, ALU.add)
        TT(s("g1"), rjk, s("g0"), ALU.mult)
        TT(s("g0"), s("g0"), rjk, ALU.add)
        TT(s("g1"), s("g1"), s("p_ijik"), ALU.add)
        TT(s("g2"), s("p_ijik"), rjk, ALU.mult)
        TT(s("gs"), s("g0"), s("g0"), ALU.mult)
        TT(s("tq"), s("g1"), s("g1"), ALU.mult)
        TT(s("gs"), s("gs"), s("tq"), ALU.add)
        TT(s("tq"), s("g2"), s("g2"), ALU.mult)
        TT(s("gs"), s("gs"), s("tq"), ALU.add)
        nc.scalar.activation(s("gs"), s("gs"), ACTF.Sqrt)
        nc.vector.tensor_scalar(out=s("gs"), in0=s("gs"), scalar1=EPS,
                                scalar2=None, op0=ALU.add)
        nc.vector.reciprocal(out=s("gs"), in_=s("gs"))
        TT(fbuf[:, sl, 0], s("g0"), s("gs"), ALU.mult)
        TT(fbuf[:, sl, 1], s("g1"), s("gs"), ALU.mult)
        TT(fbuf[:, sl, 2], s("g2"), s("gs"), ALU.mult)

        # chem features -> fbuf[:, :, 3:9]
        nc.gpsimd.tensor_tensor(out=s("zs"), in0=zj, in1=zk, op=ALU.add)
        TT(s("csum"), s("c_j"), s("c_k"), ALU.add)
        nc.gpsimd.tensor_tensor(out=s("zp"), in0=zj, in1=zk, op=ALU.mult)
        TT(s("cp"), s("c_j"), s("c_k"), ALU.mult)
        TT(s("zc"), zj, s("c_k"), ALU.mult)
        TT(s("t4"), s("c_j"), zk, ALU.mult)
        TT(s("zc"), s("zc"), s("t4"), ALU.add)
        TT(s("AA"), s("zp"), s("cp"), ALU.subtract)
        TT(s("ch0"), zi, s("zs"), ALU.add)
        TT(s("ch1"), s("c_i"), s("csum"), ALU.add)
        TT(s("ch2"), zi, s("zs"), ALU.mult)
        TT(s("ch2"), s("ch2"), s("zp"), ALU.add)
        TT(s("t4"), s("c_i"), s("csum"), ALU.mult)
        TT(s("ch2"), s("ch2"), s("t4"), ALU.subtract)
        TT(s("ch2"), s("ch2"), s("cp"), ALU.subtract)
        TT(s("ch3"), zi, s("csum"), ALU.mult)
        TT(s("t4"), s("c_i"), s("zs"), ALU.mult)
        TT(s("ch3"), s("ch3"), s("t4"), ALU.add)
        TT(s("ch3"), s("ch3"), s("zc"), ALU.add)
        TT(s("ch4"), zi, s("AA"), ALU.mult)
        TT(s("t4"), s("c_i"), s("zc"), ALU.mult)
        TT(s("ch4"), s("ch4"), s("t4"), ALU.subtract)
        TT(s("ch5"), zi, s("zc"), ALU.mult)
        TT(s("t4"), s("c_i"), s("AA"), ALU.mult)
        TT(s("ch5"), s("ch5"), s("t4"), ALU.add)
        TT(s("cs"), s("ch0"), s("ch0"), ALU.mult)
        for i in range(1, 6):
            TT(s("t4"), s(f"ch{i}"), s(f"ch{i}"), ALU.mult)
            TT(s("cs"), s("cs"), s("t4"), ALU.add)
        nc.scalar.activation(s("cs"), s("cs"), ACTF.Sqrt)
        nc.vector.tensor_scalar(out=s("cs"), in0=s("cs"), scalar1=EPS,
                                scalar2=None, op0=ALU.add)
        nc.vector.reciprocal(out=s("cs"), in_=s("cs"))
        for i in range(6):
            TT(fbuf[:, sl, 3 + i], s(f"ch{i}"), s("cs"), ALU.mult)

    def wchain_half(h):
        """Cutoff weight w = fc(rij)*fc(rik)*mask on gpsimd + scalar.
        cos(pi*d/C) = sin(pi/2 - pi*d/C), d clamped to [0, C] to stay in the
        scalar engine's sin spline range (out-of-cutoff values are masked)."""
        sl = slice(64 * h, 64 * h + 64)
        rij, rik = rij_f[:, sl], rik_f[:, sl]
        fci, fck, m1 = M["fci"][:, sl], M["fck"][:, sl], M["m1"][:, sl]
        w = w_tm[:, sl]
        g = nc.vector
        g.tensor_scalar(out=fci, in0=rij, scalar1=CUTOFF, scalar2=None,
                        op0=ALU.min)
        g.tensor_scalar(out=fck, in0=rik, scalar1=CUTOFF, scalar2=None,
                        op0=ALU.min)
        nc.scalar.activation(fci, fci, ACTF.Sin, bias=halfpi[:, 0:1],
                             scale=-PI / CUTOFF)
        nc.scalar.activation(fck, fck, ACTF.Sin, bias=halfpi[:, 0:1],
                             scale=-PI / CUTOFF)
        g.tensor_scalar(out=fci, in0=fci, scalar1=0.5, scalar2=0.5,
                        op0=ALU.mult, op1=ALU.add)
        g.tensor_scalar(out=fck, in0=fck, scalar1=0.5, scalar2=0.5,
                        op0=ALU.mult, op1=ALU.add)
        g.tensor_tensor(out=w, in0=fci, in1=fck, op=ALU.mult)
        g.tensor_scalar(out=m1, in0=rij, scalar1=CUTOFF, scalar2=None,
                        op0=ALU.is_lt)
        g.scalar_tensor_tensor(out=m1, in0=rik, scalar=CUTOFF, in1=m1,
                               op0=ALU.is_lt, op1=ALU.mult)
        g.scalar_tensor_tensor(out=m1, in0=rij, scalar=0.0, in1=m1,
                               op0=ALU.is_gt, op1=ALU.mult)
        g.scalar_tensor_tensor(out=m1, in0=rik, scalar=0.0, in1=m1,
                               op0=ALU.is_gt, op1=ALU.mult)
        g.tensor_tensor(out=w, in0=w, in1=m1, op=ALU.mult)

    # ---- feature-major activations via 32x32 block transpose ----
    xfm_r = big.tile([128, 4096], MM_DT, tag="xfm_r")
    # xb3 columns are block-major: col = 128*bb + 32*a + u  (token 4096a+32bb+u)
    # so each col-block's 128 tokens are contiguous for the L6 stationary AP.
    xb3 = big.tile([128, 16384], MM_DT, tag="xb3")
    xb3_4 = xb3[:].rearrange("p (bb a u) -> p bb a u", a=4, u=32)

    ga = gap.tile([128, 256], F32, tag="ga")

    w6 = cb["w6"]
    ones128 = cb["ones128"]
    b6mul = cb["b6div2"]
    oh = oh_full

    def mm_pair(dst_ps, wt, src, tloc=None):
        for j in range(2):
            rhs = (xfm_r[:, 512 * (tloc + j):512 * (tloc + j) + 512]
                   if src is None else src[:, 512 * j:512 * j + 512])
            nc.tensor.matmul(dst_ps[:, 512 * j:512 * j + 512], _mm(wt[:]),
                             _mm(rhs), start=True, stop=True)

    def make_mlp_stages(s):
        """Stage callables for supertile s (tiles {2s, 2s+1, 2s+16, 2s+17}).

        Pair-members are packed via block-stacked stationary weights: one
        K=128 matmul computes member 0 into out rows 0-63 and member 1 into
        rows 64-127 (rows outside each member's block are zero weights).
        Split into stages so two supertiles + L6 work interleave on the PE
        (keeps it busy across each tanh dependency, which also keeps the
        HAM clock gate at full rate)."""
        tloc = (2 * s) % 8                 # within-strip tile index base
        st = {}

        def tanh_stage(ps, bname):
            dst = actp.tile([128, 1024], MM_DT, tag="h", name="h")
            nc.scalar.activation(dst[:], ps[:], ACTF.Tanh,
                                 bias=cb[bname][:, 0:1])
            return dst

        def st_l0():
            ps = psp.tile([128, 1024], F32, tag="ps", name="ps")
            w0s = cb["w0s0"] if s // 4 == 0 else cb["w0s1"]
            mm_pair(ps, w0s, None, tloc)
            st["xres"] = tanh_stage(ps, "b0rep")

        def st_l1():
            ps = psp.tile([128, 1024], F32, tag="ps", name="ps")
            mm_pair(ps, cb["w1d"], st["xres"])
            x1 = tanh_stage(ps, "b1rep")
            xb1 = actp.tile([128, 1024], MM_DT, tag="h", name="h")
            nc.vector.tensor_tensor(out=xb1[:], in0=x1[:], in1=st["xres"][:],
                                    op=ALU.add)
            st["xb1"] = xb1

        def st_l2():
            ps = psp.tile([128, 1024], F32, tag="ps", name="ps")
            mm_pair(ps, cb["w2d"], st["xb1"])
            st["x2"] = tanh_stage(ps, "b2rep")

        def st_l3():
            ps = psp.tile([128, 1024], F32, tag="ps", name="ps")
            mm_pair(ps, cb["w3d"], st["x2"])
            st["x3"] = tanh_stage(ps, "b3rep")

        def st_l4():
            ps = psp.tile([128, 1024], F32, tag="ps", name="ps")
            mm_pair(ps, cb["w4d"], st["x3"])
            x4 = tanh_stage(ps, "b4rep")
            xb2 = actp.tile([128, 1024], MM_DT, tag="h", name="h")
            nc.vector.tensor_tensor(out=xb2[:], in0=x4[:], in1=st["xb1"][:],
                                    op=ALU.add)
            st["xb2"] = xb2

        def st_l5(m):
            ps5 = psp.tile([128, 1024], F32, tag="ps", name="ps")
            mm_pair(ps5, cb["w5p0"] if m == 0 else cb["w5p1"], st["xb2"])
            a = (s // 4) + 2 * m
            nc.scalar.activation(
                xb3_4[:, 16 * tloc:16 * tloc + 32, a, :], ps5[:], ACTF.Tanh,
                bias=cb["b5c"][:, 0:1])

        return [st_l0, st_l1, st_l2, st_l3, st_l4,
                lambda: st_l5(0), lambda: st_l5(1)]

    def l6_quad(q):
        """Final layer + weighted accumulation for col-blocks 4q..4q+3."""
        ps6 = psp.tile([128, 1024], F32, tag="ps")
        for jj in range(2):
            nc.tensor.matmul(
                ps6[:, 512 * jj:512 * jj + 512],
                _mm(ones128[0:1, :]), _mm(b6mul[0:1, :]),
                start=True, stop=False, skip_group_check=True,
            )
        for i in range(4):
            b_ = 4 * q + i
            nc.tensor.matmul(
                ps6[:, 256 * i:256 * i + 256],
                _mm(xb3[:, 128 * b_:128 * b_ + 128]),
                _mm(w6[:]),
                start=False, stop=True, skip_group_check=True,
            )
        otm = otmp.tile([128, 1024], MM_DT, tag="otm")
        nc.scalar.activation(otm[:], ps6[:], ACTF.Tanh)
        ab = apool.tile([128, 4, 32], MM_DT, tag="ab", name="ab")
        oh4 = oh[:, 128 * q:128 * q + 128].rearrange("p (b u) -> p b u", b=4)
        wq = w_tm[:, 4 * q:4 * q + 4]
        w_bc = bass.AP(tensor=wq.tensor, offset=wq.offset,
                       ap=[list(wq.ap[0]), [int(wq.ap[1][0]), 4], [0, 32]])
        nc.vector.tensor_tensor(out=ab[:], in0=oh4, in1=w_bc, op=ALU.mult)
        for i in range(4):
            b_ = 4 * q + i
            nc.tensor.matmul(
                ga[32 * i:32 * i + 32, :], _mm(ab[:, i, :]),
                _mm(otm[:, 256 * i:256 * i + 256]),
                start=(q == 0), stop=(q == 31), skip_group_check=True,
                tile_position=(0, 32 * i),
            )

    # Feature pipeline: compute half the col-blocks, transpose them in
    # place (32x32 blocks fill-then-drain, safe in place), convert to the
    # matmul dtype, then the MLP starts while the second half computes.
    # The cutoff-weight chain runs on gpsimd/scalar off the critical path.
    for h in range(2):
        features_half(h)
        for c in range(4 * h, 4 * h + 4):
            nc.vector.transpose(out=fbuf[:, 16 * c:16 * c + 16, :],
                                in_=fbuf[:, 16 * c:16 * c + 16, :])
            nc.vector.tensor_copy(xfm_r[:, 512 * c:512 * c + 512],
                                  fbuf[:, 16 * c:16 * c + 16, :])
        wchain_half(h)

    # Interleave: each group emits two supertile stage-chains round-robin
    # with the PREVIOUS group's L6 quads, so the PE always has independent
    # matmul work while tanh stages drain.
    pending_quads = []
    for g in range(4):
        sa = make_mlp_stages(g)
        sb = make_mlp_stages(g + 4)
        quads = list(pending_quads)
        for i in range(7):
            sa[i]()
            sb[i]()
            if i < len(quads):
                l6_quad(quads[i])
        for q in quads[7:]:
            l6_quad(q)
        pending_quads = list(range(8 * g, 8 * g + 8))
    for q in pending_quads:
        l6_quad(q)

    # ---- sum the 4 GA strips, normalize rows, write out ----
    ga4sb = fmaps.tile([128, 256], F32, tag="ga4sb", name="ga4sb")
    nc.scalar.activation(ga4sb[:], ga[:], ACTF.Copy)
    gsum = gap.tile([32, 256], F32, tag="gsum", name="gsum")
    nc.tensor.matmul(gsum[:, :], staged["wpack"][:, 1926:1958],
                     ga4sb[:], start=True, stop=True)
    ga = gsum
    sqbuf = fmaps.tile([32, 256], F32, tag="sqbuf")
    ss = fmaps.tile([32, 1], F32, tag="ss")
    nc.scalar.activation(sqbuf[:], ga[:], ACTF.Square, accum_out=ss[:, 0:1])
    nc.scalar.activation(ss[:], ss[:], ACTF.Sqrt)
    nc.vector.tensor_scalar(out=ss[:], in0=ss[:], scalar1=EPS, scalar2=None,
                            op0=ALU.add)
    nc.vector.reciprocal(out=ss[:], in_=ss[:])
    outs = fmaps.tile([32, 256], F32, tag="outs")
    nc.vector.tensor_scalar(out=outs[:], in0=ga[:], scalar1=ss[:, 0:1],
                            scalar2=None, op0=ALU.mult)
    nc.sync.dma_start(out_ap[:], outs[:])


# --------------------------------------------------------------------------
# module build + run
# --------------------------------------------------------------------------

_BUILT = {}


def build_bass():
    key = _MM_DT_NAME
    if key in _BUILT:
        return _BUILT[key]
    nc = bacc.Bacc(trn_type="TRN2", target_bir_lowering=False, debug=False)
    ins = {
        name: nc.dram_tensor(name, shape,
                             {"float32": F32, "bfloat16": BF16}[dtname],
                             kind="ExternalInput").ap()
        for name, (shape, is_mm, dtname) in INPUT_SPECS.items()
    }
    out = nc.dram_tensor("out", [NA, 256], F32, kind="ExternalOutput").ap()
    with tile.TileContext(nc) as tc:
        with ExitStack() as ctx:
            build_kernel(ctx, tc, out, ins)
    nc.finalize()
    _BUILT[key] = nc
    return nc


def _run(inputs, **spmd_kwargs):
    D = np.asarray(inputs["distance_matrices"], np.float32)
    S = np.asarray(inputs["num_species_batch"], np.float32)
    Ws = [np.asarray(inputs[f"W{i}"], np.float32) for i in range(7)]
    bs = [np.asarray(inputs[f"b{i}"], np.float32) for i in range(7)]
    nc = build_bass()
    in_maps = [make_core_inputs(D[c], S[c], Ws, bs) for c in range(B)]
    res = run_bass_kernel_spmd(nc, in_maps, core_ids=list(range(B)),
                               **spmd_kwargs)
    out = np.stack([res.results[c]["out"] for c in range(B)], axis=0)
    return out.astype(np.float32), res


def kernel(**inputs):
    out, _ = _run(inputs)
    return out
